# revision 7
# baseline (speedup 1.0000x reference)
"""Causal self-attention (B=2, T=2048, C=1024, H=16, Dh=64) on 8 trn2 NeuronCores.

Sharding: 2-way data-parallel over batch x 4-way tensor-parallel over heads.
Core c handles batch b=c//4 and heads 4g..4g+3 where g=c%4:
  - computes q,k (transposed layout) and v for its 4 heads,
  - causal attention per head entirely on-chip,
  - row-parallel output projection against w_proj[:, g*256:(g+1)*256],
  - returns the partial [T, C] projection; host sums the 4 partials per batch.

Schedule notes (hard-won on hardware):
  - PSUM accumulation groups are bank-granular: never interleave two
    accumulation groups in one 2KB bank.
  - The PE drops p-state on idle gaps; keep its instruction stream dense and
    keep the score->exp->PV chain strictly PE/ACT (mask via PE matmul
    accumulation, not a third engine).
  - v-projection runs k-outer across all 8 banks (2 waves) so the PE starts
    as soon as the first x chunk lands from HBM.
  - attention is chunk-outer (j) per head; scores for two consecutive
    k-tiles live in one double-bank PSUM tile so exp is one wide ACT op.
  - softmax denominators come from all-ones stationary columns; V/ones
    halves are swapped per head parity so PV output and denominator land on
    the lanes the otj layout wants; 1/l is one fast custom-DVE reciprocal on
    the replicated rows plus one cross-lane DMA.
  - chunk-outer holds only ~2 otp banks, so the output projection gets a
    dedicated 2-bank ring and streams per-chunk inside head 3, overlapping
    the 8MB output DMA with compute.
"""

import numpy as np
import ml_dtypes
from contextlib import ExitStack

import concourse.bass as bass
import concourse.tile as tile
from concourse import bacc, mybir, bass_utils

F32 = mybir.dt.float32
BF16 = mybir.dt.bfloat16

T = 2048
C = 1024
HL = 4  # local heads per core
DH = 64
NKT = T // 128  # 16 k-tiles
NQ = T // 512  # 4 q-chunks
NCC = C // 128  # 8 contraction chunks


def _pin_act_table():
    """Restrict the activation-table registry to a single set containing Exp
    so activation ops never reload tables."""
    import concourse.bacc as bacc_mod
    from concourse.hw_specs import get_activation_tables as real

    def only_combined(arch):
        t = real(arch)
        name = "natural_log_exp_and_others"
        if name in t:
            return {name: t[name]}
        return t

    bacc_mod.get_activation_tables = only_combined


def build_nc():
    nc = bacc.Bacc("TRN2", target_bir_lowering=False, debug=False)
    xt_d = nc.dram_tensor("xt", [C, T], BF16, kind="ExternalInput").ap()
    wqkt_d = nc.dram_tensor("wqkt", [C, 512], BF16, kind="ExternalInput").ap()
    wvt_d = nc.dram_tensor("wvt", [C, 256], BF16, kind="ExternalInput").ap()
    wpt_d = nc.dram_tensor("wpt", [256, C], BF16, kind="ExternalInput").ap()
    p_d = nc.dram_tensor("p", [T, C], F32, kind="ExternalOutput").ap()

    with tile.TileContext(nc) as tc:
        with ExitStack() as ctx:
            _body(ctx, tc, xt_d, wqkt_d, wvt_d, wpt_d, p_d)
    nc.compile()
    return nc


def _body(ctx, tc, xt_d, wqkt_d, wvt_d, wpt_d, p_d):
    nc = tc.nc
    Exp = mybir.ActivationFunctionType.Exp

    persist = ctx.enter_context(tc.tile_pool(name="persist", bufs=1))
    ptp = ctx.enter_context(tc.tile_pool(name="ptp", bufs=4))
    rrp = ctx.enter_context(tc.tile_pool(name="rrp", bufs=4))
    pout = ctx.enter_context(tc.tile_pool(name="pout", bufs=3))
    pp = ctx.enter_context(tc.tile_pool(name="pp", bufs=1, space="PSUM"))

    # ---- persistent SBUF tiles ----
    identb = persist.tile([128, 128], BF16, tag="identb")
    maskb = persist.tile([128, 512], BF16, tag="maskb")
    wqkT = persist.tile([128, NCC, 512], BF16, tag="wqkT")
    wvT = persist.tile([128, NCC, 256], BF16, tag="wvT")
    wpT = persist.tile([128, 2, C], BF16, tag="wpT")
    xT = [persist.tile([128, T], BF16, tag=f"xT{k}", name=f"xT{k}")
          for k in range(NCC)]
    qkT = [persist.tile([128, T], BF16, tag=f"qkT{m}", name=f"qkT{m}")
           for m in range(4)]
    # vs[:, i, h*128:(h+1)*128]: per k-tile i, head h: [v(64) | ones(64)] for
    # even h, [ones(64) | v(64)] for odd h, so PV outputs / denominators land
    # on the lanes where the otj layout wants them.
    vs = persist.tile([128, NKT, HL * 128], BF16, tag="vs")
    otj = [persist.tile([128, 2, 512], BF16, tag=f"otj{j}", name=f"otj{j}")
           for j in range(NQ)]

    # ---- input DMAs (sync queue: x + wv; gpsimd queue: wqk + wp) ----
    for k in range(NCC):
        nc.sync.dma_start(wvT[:, k, :], wvt_d[k * 128:(k + 1) * 128, :])
        nc.sync.dma_start(xT[k][:], xt_d[k * 128:(k + 1) * 128, :])
    for k in range(NCC):
        nc.gpsimd.dma_start(wqkT[:, k, :], wqkt_d[k * 128:(k + 1) * 128, :])
    for c in range(2):
        nc.gpsimd.dma_start(wpT[:, c, :], wpt_d[c * 128:(c + 1) * 128, :])

    # ones everywhere in vs; the v-projection copies overwrite the v halves.
    nc.gpsimd.memset(vs[:], 1.0)
    # bf16 identity (for PSUM-accumulate mask adds) and the causal band mask:
    # maskb[k, q] = 0 where q >= k else -30000 (additive, pre-exp).
    nc.gpsimd.memset(identb[:], 0.0)
    nc.gpsimd.affine_select(
        out=identb[:], in_=identb[:], compare_op=mybir.AluOpType.not_equal,
        fill=1.0, base=0, channel_multiplier=1, pattern=[[-1, 128]])
    nc.gpsimd.memset(maskb[:], 0.0)
    nc.gpsimd.affine_select(
        out=maskb[:], in_=maskb[:], compare_op=mybir.AluOpType.is_ge,
        fill=-30000.0, base=0, channel_multiplier=-1, pattern=[[1, 512]])

    # ---- C: v strips, k-outer in 2 waves of 8 full banks ----
    # Each C tile [128,256] owns a full bank (one accumulation group/bank).
    def emit_c_wave(w):
        packs = []  # (tile, [i...], [coloffs...])
        for s in range(2):
            t2 = pp.tile([128, 1024], F32, tag="st2", bufs=2)
            packs.append((t2, [8 * w + 2 * s, 8 * w + 2 * s + 1], [0, 512]))
        for s in range(2):
            t1 = pp.tile([128, 512], F32, tag="ot", bufs=2)
            packs.append((t1, [8 * w + 4 + s], [0]))
        for s in range(2):
            t1 = pp.tile([128, 512], F32, tag="pj", bufs=2)
            packs.append((t1, [8 * w + 6 + s], [0]))
        for k in range(NCC):
            for (pt_, iis, offs) in packs:
                for i, off in zip(iis, offs):
                    nc.tensor.matmul(
                        pt_[:, off:off + 256],
                        lhsT=xT[k][:, i * 128:(i + 1) * 128],
                        rhs=wvT[:, k, :],
                        start=(k == 0), stop=(k == NCC - 1))
        for (pt_, iis, offs) in packs:
            i0 = iis[0]
            if len(iis) == 2:
                src = pt_[:].rearrange("p (i r) -> p i r", i=2)[
                    :, :, 0:256].rearrange("p i (h d) -> p i h d", h=4)
                dst = vs[:, i0:i0 + 2, :].rearrange(
                    "p i (h e) -> p i h e", h=4)
                for par in range(2):
                    nc.vector.tensor_copy(
                        dst[:, :, par::2, par * 64:(par + 1) * 64],
                        src[:, :, par::2, :])
            else:
                src = pt_[:, 0:256].rearrange("p (h d) -> p h d", h=4)
                dst = vs[:, i0, :].rearrange("p (h e) -> p h e", h=4)
                for par in range(2):
                    nc.vector.tensor_copy(
                        dst[:, par::2, par * 64:(par + 1) * 64],
                        src[:, par::2, :])

    # ---- B: qkT[m] = (wqk @ x.T) block rows, paired 512-col banks ----
    def emit_b(m):
        for np_ in range(2):
            st = pp.tile([128, 1024], F32, tag="st2", bufs=2)
            for k in range(NCC):
                for half in range(2):
                    nc.tensor.matmul(
                        st[:, half * 512:(half + 1) * 512],
                        lhsT=wqkT[:, k, m * 128:(m + 1) * 128],
                        rhs=xT[k][:, np_ * 1024 + half * 512:
                                  np_ * 1024 + (half + 1) * 512],
                        start=(k == 0), stop=(k == NCC - 1))
            nc.vector.tensor_copy(
                qkT[m][:, np_ * 1024:(np_ + 1) * 1024], st[:])

    # ---- attention: chunk-outer per head, PE-side causal mask ----
    def emit_head(h, post_j=None):
        part = (h % 2) * 64
        qt = qkT[h // 2]
        kt = qkT[2 + h // 2]
        groups = []  # (j, [i, i+1])
        for j in range(NQ):
            iis = list(range(0, 4 * j + 4))
            for a0 in range(0, len(iis), 2):
                groups.append((j, iis[a0:a0 + 2]))
        otps = {}
        pend = {}

        def scores(gi):
            j, iis = groups[gi]
            st = pp.tile([128, 1024], F32, tag="st2", bufs=2)
            ptt = ptp.tile([128, 1024], BF16, tag="pt")
            segs = []
            for idx, i in enumerate(iis):
                d = i - 4 * j
                co = 128 * d if d > 0 else 0
                base = idx * 512
                diag = i >= 4 * j
                nc.tensor.matmul(
                    st[:, base + co:base + 512],
                    lhsT=kt[part:part + 64, i * 128:(i + 1) * 128],
                    rhs=qt[part:part + 64, j * 512 + co:(j + 1) * 512],
                    start=True, stop=not diag)
                if diag:  # accumulate -30000 over the k>q triangle on the PE
                    nc.tensor.matmul(
                        st[:, base + co:base + 512],
                        lhsT=identb[:],
                        rhs=maskb[:, 0:512 - co],
                        start=False, stop=True)
                segs.append((base + co, base + 512, i, d))
            # exp over maximal contiguous column runs
            k0 = 0
            while k0 < len(segs):
                lo, hi = segs[k0][0], segs[k0][1]
                k1 = k0 + 1
                while k1 < len(segs) and segs[k1][0] == hi:
                    hi = segs[k1][1]
                    k1 += 1
                nc.scalar.activation(ptt[:, lo:hi], st[:, lo:hi], Exp)
                k0 = k1
            pend[gi] = (ptt, segs)

        def pvs(gi):
            j, iis = groups[gi]
            ptt, segs = pend.pop(gi)
            if j not in otps:
                otps[j] = pp.tile([128, 512], F32, tag="ot", bufs=2,
                                  name=f"otp{h}_{j}")
            otp = otps[j]
            for (lo, hi, i, d) in segs:
                co = 128 * d if d > 0 else 0
                nc.tensor.matmul(
                    otp[:, co:512],
                    lhsT=vs[:, i, h * 128:(h + 1) * 128],
                    rhs=ptt[:, lo:hi],
                    start=(i == 0), stop=(i == 4 * j + 3))

        def normalize(j):
            # custom-DVE ops (recip) only work at partition base 0; standard
            # DVE ops take per-operand partition bases, so cross-base moves
            # ride on tensor_copy / the mul's in1.
            otp = otps.pop(j)
            cc = h // 2
            lbA = rrp.tile([64, 512], F32, tag="lbA")
            if h % 2 == 0:
                # otp rows: [O(0:64) | l(64:128)]
                lc = rrp.tile([64, 512], F32, tag="lc")
                nc.vector.tensor_copy(lc[:], otp[64:128, :])
                nc.vector.reciprocal_approx_fast(lbA[:], lc[:])
                nc.vector.tensor_mul(
                    otj[j][0:64, cc, :], otp[0:64, :], lbA[:])
            else:
                # otp rows: [l(0:64) | O(64:128)]
                nc.vector.reciprocal_approx_fast(lbA[:], otp[0:64, :])
                nc.vector.tensor_mul(
                    otj[j][64:128, cc, :], otp[64:128, :], lbA[:])

        scores(0)
        for gi in range(1, len(groups)):
            scores(gi)
            pvs(gi - 1)
            if groups[gi][0] != groups[gi - 1][0]:
                jdone = groups[gi - 1][0]
                normalize(jdone)
                if post_j is not None:
                    post_j(jdone)
        pvs(len(groups) - 1)
        normalize(groups[-1][0])
        if post_j is not None:
            post_j(groups[-1][0])

    # ---- output projection: per chunk, own 2-bank ring ----
    def emit_proj(j):
        for tbl in range(4):
            po = pout.tile([128, C], F32, tag="po")
            for n2 in range(2):
                ps = pp.tile([128, 512], F32, tag="pj", bufs=2)
                for cc in range(2):
                    nc.tensor.matmul(
                        ps[:],
                        lhsT=otj[j][:, cc, tbl * 128:(tbl + 1) * 128],
                        rhs=wpT[:, cc, n2 * 512:(n2 + 1) * 512],
                        start=(cc == 0), stop=(cc == 1))
                nc.vector.tensor_copy(po[:, n2 * 512:(n2 + 1) * 512], ps[:])
            tb = 4 * j + tbl
            nc.sync.dma_start(p_d[tb * 128:(tb + 1) * 128, :], po[:])

    emit_c_wave(0)
    emit_c_wave(1)
    emit_b(0)
    emit_b(2)
    emit_head(0)
    emit_head(1)
    emit_b(1)
    emit_b(3)
    emit_head(2)
    emit_head(3, post_j=emit_proj)


_NC_CACHE = None


def _get_nc():
    global _NC_CACHE
    if _NC_CACHE is None:
        _NC_CACHE = build_nc()
    return _NC_CACHE


def make_in_maps(x, w_qkv, w_proj):
    x = np.asarray(x, np.float32)
    w_qkv = np.asarray(w_qkv, np.float32)
    w_proj = np.asarray(w_proj, np.float32)
    bf = ml_dtypes.bfloat16
    in_maps = []
    for c in range(8):
        b, g = divmod(c, 4)
        wq = w_qkv[g * 256:(g + 1) * 256] * 0.125  # fold 1/sqrt(Dh)
        wk = w_qkv[C + g * 256:C + (g + 1) * 256]
        wv = w_qkv[2 * C + g * 256:2 * C + (g + 1) * 256]
        wqk = np.concatenate([wq, wk], 0)  # [512, C]
        in_maps.append({
            "xt": np.ascontiguousarray(x[b].T).astype(bf),
            "wqkt": np.ascontiguousarray(wqk.T).astype(bf),
            "wvt": np.ascontiguousarray(wv.T).astype(bf),
            "wpt": np.ascontiguousarray(w_proj[:, g * 256:(g + 1) * 256].T).astype(bf),
        })
    return in_maps


def combine(results):
    return np.stack(
        [results[4 * b]["p"] + results[4 * b + 1]["p"]
         + results[4 * b + 2]["p"] + results[4 * b + 3]["p"]
         for b in range(2)], 0)


def kernel(x, w_qkv, w_proj):
    nc = _get_nc()
    res = bass_utils.run_bass_kernel_spmd(
        nc, make_in_maps(x, w_qkv, w_proj), core_ids=list(range(8)))
    return combine(res.results)


# revision 9
# speedup vs baseline: 1.0766x; 1.0766x over previous
"""Causal self-attention (B=2, T=2048, C=1024, H=16, Dh=64) on 8 trn2 NeuronCores.

Sharding: 2-way data-parallel over batch x 4-way tensor-parallel over heads.
Core c handles batch b=c//4 and heads 4g..4g+3 where g=c%4:
  - computes q,k (transposed layout) and v for its 4 heads,
  - causal attention per head entirely on-chip,
  - row-parallel output projection against w_proj[:, g*256:(g+1)*256],
  - returns the partial [T, C] projection; host sums the 4 partials per batch.

Schedule notes (hard-won on hardware):
  - PSUM accumulation groups are bank-granular: never interleave two
    accumulation groups in one 2KB bank.
  - The PE drops p-state on idle gaps; keep its instruction stream dense and
    keep the score->exp->PV chain strictly PE/ACT (mask via PE matmul
    accumulation, not a third engine).
  - v-projection runs k-outer across all 8 banks (2 waves) so the PE starts
    as soon as the first x chunk lands from HBM.
  - attention is chunk-outer (j) per head; scores for two consecutive
    k-tiles live in one double-bank PSUM tile so exp is one wide ACT op.
  - softmax denominators come from all-ones stationary columns; V/ones
    halves are swapped per head parity so PV output and denominator land on
    the lanes the otj layout wants; 1/l is one fast custom-DVE reciprocal on
    the replicated rows plus one cross-lane DMA.
  - chunk-outer holds only ~2 otp banks, so the output projection gets a
    dedicated 2-bank ring and streams per-chunk inside head 3, overlapping
    the 8MB output DMA with compute.
"""

import numpy as np
import ml_dtypes
from contextlib import ExitStack

import concourse.bass as bass
import concourse.tile as tile
from concourse import bacc, mybir, bass_utils

F32 = mybir.dt.float32
BF16 = mybir.dt.bfloat16

T = 2048
C = 1024
HL = 4  # local heads per core
DH = 64
NKT = T // 128  # 16 k-tiles
NQ = T // 512  # 4 q-chunks
NCC = C // 128  # 8 contraction chunks


def _pin_act_table():
    """Restrict the activation-table registry to a single set containing Exp
    so activation ops never reload tables."""
    import concourse.bacc as bacc_mod
    from concourse.hw_specs import get_activation_tables as real

    def only_combined(arch):
        t = real(arch)
        name = "natural_log_exp_and_others"
        if name in t:
            return {name: t[name]}
        return t

    bacc_mod.get_activation_tables = only_combined


def build_nc():
    nc = bacc.Bacc("TRN2", target_bir_lowering=False, debug=False)
    xt_d = nc.dram_tensor("xt", [C, T], BF16, kind="ExternalInput").ap()
    wqkt_d = nc.dram_tensor("wqkt", [C, 512], BF16, kind="ExternalInput").ap()
    wvt_d = nc.dram_tensor("wvt", [C, 256], BF16, kind="ExternalInput").ap()
    wpt_d = nc.dram_tensor("wpt", [256, C], BF16, kind="ExternalInput").ap()
    p_d = nc.dram_tensor("p", [T, C], F32, kind="ExternalOutput").ap()

    with tile.TileContext(nc) as tc:
        with ExitStack() as ctx:
            _body(ctx, tc, xt_d, wqkt_d, wvt_d, wpt_d, p_d)
    nc.compile()
    return nc


def _body(ctx, tc, xt_d, wqkt_d, wvt_d, wpt_d, p_d):
    nc = tc.nc
    Exp = mybir.ActivationFunctionType.Exp

    persist = ctx.enter_context(tc.tile_pool(name="persist", bufs=1))
    ptp = ctx.enter_context(tc.tile_pool(name="ptp", bufs=4))
    rrp = ctx.enter_context(tc.tile_pool(name="rrp", bufs=4))
    pout = ctx.enter_context(tc.tile_pool(name="pout", bufs=3))
    pp = ctx.enter_context(tc.tile_pool(name="pp", bufs=1, space="PSUM"))

    # ---- persistent SBUF tiles ----
    identb = persist.tile([128, 128], BF16, tag="identb")
    maskb = persist.tile([128, 512], BF16, tag="maskb")
    wqkT = persist.tile([128, NCC, 512], BF16, tag="wqkT")
    wvT = persist.tile([128, NCC, 256], BF16, tag="wvT")
    wpT = persist.tile([128, 2, C], BF16, tag="wpT")
    xT = [persist.tile([128, T], BF16, tag=f"xT{k}", name=f"xT{k}")
          for k in range(NCC)]
    qkT = [persist.tile([128, T], BF16, tag=f"qkT{m}", name=f"qkT{m}")
           for m in range(4)]
    # vs[:, i, h*128:(h+1)*128]: per k-tile i, head h: [v(64) | ones(64)] for
    # even h, [ones(64) | v(64)] for odd h, so PV outputs / denominators land
    # on the lanes where the otj layout wants them.
    vs = persist.tile([128, NKT, HL * 128], BF16, tag="vs")
    otj = [persist.tile([128, 2, 512], BF16, tag=f"otj{j}", name=f"otj{j}")
           for j in range(NQ)]

    # ---- input DMAs (sync queue: x + wv; gpsimd queue: wqk + wp) ----
    for k in range(NCC):
        nc.sync.dma_start(wvT[:, k, :], wvt_d[k * 128:(k + 1) * 128, :])
        nc.sync.dma_start(xT[k][:], xt_d[k * 128:(k + 1) * 128, :])
    for k in range(NCC):
        nc.gpsimd.dma_start(wqkT[:, k, :], wqkt_d[k * 128:(k + 1) * 128, :])
    for c in range(2):
        nc.gpsimd.dma_start(wpT[:, c, :], wpt_d[c * 128:(c + 1) * 128, :])

    # ones everywhere in vs; the v-projection copies overwrite the v halves.
    nc.gpsimd.memset(vs[:], 1.0)
    # bf16 identity (for PSUM-accumulate mask adds) and the causal band mask:
    # maskb[k, q] = 0 where q >= k else -30000 (additive, pre-exp).
    nc.gpsimd.memset(identb[:], 0.0)
    nc.gpsimd.affine_select(
        out=identb[:], in_=identb[:], compare_op=mybir.AluOpType.not_equal,
        fill=1.0, base=0, channel_multiplier=1, pattern=[[-1, 128]])
    nc.gpsimd.memset(maskb[:], 0.0)
    nc.gpsimd.affine_select(
        out=maskb[:], in_=maskb[:], compare_op=mybir.AluOpType.is_ge,
        fill=-30000.0, base=0, channel_multiplier=-1, pattern=[[1, 512]])

    # ---- C: v strips, k-outer in 2 waves of 8 full banks ----
    # Each C tile [128,256] owns a full bank (one accumulation group/bank).
    def emit_c_wave(w):
        packs = []  # (tile, [i...], [coloffs...])
        for s in range(2):
            t2 = pp.tile([128, 1024], F32, tag="st2", bufs=2)
            packs.append((t2, [8 * w + 2 * s, 8 * w + 2 * s + 1], [0, 512]))
        for s in range(2):
            t1 = pp.tile([128, 512], F32, tag="ot", bufs=2)
            packs.append((t1, [8 * w + 4 + s], [0]))
        for s in range(2):
            t1 = pp.tile([128, 512], F32, tag="pj", bufs=2)
            packs.append((t1, [8 * w + 6 + s], [0]))
        for k in range(NCC):
            for (pt_, iis, offs) in packs:
                for i, off in zip(iis, offs):
                    nc.tensor.matmul(
                        pt_[:, off:off + 256],
                        lhsT=xT[k][:, i * 128:(i + 1) * 128],
                        rhs=wvT[:, k, :],
                        start=(k == 0), stop=(k == NCC - 1))
        for (pt_, iis, offs) in packs:
            i0 = iis[0]
            if len(iis) == 2:
                src = pt_[:].rearrange("p (i r) -> p i r", i=2)[
                    :, :, 0:256].rearrange("p i (h d) -> p i h d", h=4)
                dst = vs[:, i0:i0 + 2, :].rearrange(
                    "p i (h e) -> p i h e", h=4)
                for par in range(2):
                    nc.vector.tensor_copy(
                        dst[:, :, par::2, par * 64:(par + 1) * 64],
                        src[:, :, par::2, :])
            else:
                src = pt_[:, 0:256].rearrange("p (h d) -> p h d", h=4)
                dst = vs[:, i0, :].rearrange("p (h e) -> p h e", h=4)
                for par in range(2):
                    nc.vector.tensor_copy(
                        dst[:, par::2, par * 64:(par + 1) * 64],
                        src[:, par::2, :])

    # ---- B: qkT[m] = (wqk @ x.T) block rows, paired 512-col banks ----
    def emit_b(m):
        for np_ in range(2):
            st = pp.tile([128, 1024], F32, tag="st2", bufs=2)
            for k in range(NCC):
                for half in range(2):
                    nc.tensor.matmul(
                        st[:, half * 512:(half + 1) * 512],
                        lhsT=wqkT[:, k, m * 128:(m + 1) * 128],
                        rhs=xT[k][:, np_ * 1024 + half * 512:
                                  np_ * 1024 + (half + 1) * 512],
                        start=(k == 0), stop=(k == NCC - 1))
            nc.vector.tensor_copy(
                qkT[m][:, np_ * 1024:(np_ + 1) * 1024], st[:])

    # ---- attention: chunk-major across heads, PE-side causal mask ----
    # Flat software-pipelined stream of score/exp groups; after all four
    # heads finish chunk j the output projection for j streams immediately.
    def emit_attention(post_j):
        groups = []  # (j, h, [i, i+1], last_of_chunk)
        for j in range(NQ):
            iis = list(range(0, 4 * j + 4))
            for h in range(HL):
                for a0 in range(0, len(iis), 2):
                    groups.append(
                        (j, h, iis[a0:a0 + 2], a0 + 2 >= len(iis)))
        otps = {}
        pend = {}

        def scores(gi):
            j, h, iis, _ = groups[gi]
            part = (h % 2) * 64
            qt = qkT[h // 2]
            kt = qkT[2 + h // 2]
            st = pp.tile([128, 1024], F32, tag="st2", bufs=2)
            ptt = ptp.tile([128, 1024], BF16, tag="pt")
            segs = []
            for idx, i in enumerate(iis):
                d = i - 4 * j
                co = 128 * d if d > 0 else 0
                base = idx * 512
                diag = i >= 4 * j
                nc.tensor.matmul(
                    st[:, base + co:base + 512],
                    lhsT=kt[part:part + 64, i * 128:(i + 1) * 128],
                    rhs=qt[part:part + 64, j * 512 + co:(j + 1) * 512],
                    start=True, stop=not diag)
                if diag:
                    # accumulate -30000 over the k>q triangle; only the
                    # first 128 cols of the region can be masked.
                    nc.tensor.matmul(
                        st[:, base + co:base + co + 128],
                        lhsT=identb[:],
                        rhs=maskb[:, 0:128],
                        start=False, stop=True)
                segs.append((base + co, base + 512, i, d))
            # exp over maximal contiguous column runs
            k0 = 0
            while k0 < len(segs):
                lo, hi = segs[k0][0], segs[k0][1]
                k1 = k0 + 1
                while k1 < len(segs) and segs[k1][0] == hi:
                    hi = segs[k1][1]
                    k1 += 1
                nc.scalar.activation(ptt[:, lo:hi], st[:, lo:hi], Exp)
                k0 = k1
            pend[gi] = (ptt, segs)

        def pvs(gi):
            j, h, iis, _ = groups[gi]
            ptt, segs = pend.pop(gi)
            if (j, h) not in otps:
                otps[(j, h)] = pp.tile([128, 512], F32, tag="ot", bufs=2,
                                       name=f"otp{h}_{j}")
            otp = otps[(j, h)]
            for (lo, hi, i, d) in segs:
                co = 128 * d if d > 0 else 0
                nc.tensor.matmul(
                    otp[:, co:512],
                    lhsT=vs[:, i, h * 128:(h + 1) * 128],
                    rhs=ptt[:, lo:hi],
                    start=(i == 0), stop=(i == 4 * j + 3))

        def normalize(j, h):
            # custom-DVE ops (recip) only work at partition base 0; standard
            # DVE ops take per-operand partition bases, so cross-base moves
            # ride on tensor_copy / the mul's in1.
            otp = otps.pop((j, h))
            cc = h // 2
            lbA = rrp.tile([64, 512], F32, tag="lbA")
            if h % 2 == 0:
                # otp rows: [O(0:64) | l(64:128)]
                lc = rrp.tile([64, 512], F32, tag="lc")
                nc.vector.tensor_copy(lc[:], otp[64:128, :])
                nc.vector.reciprocal_approx_fast(lbA[:], lc[:])
                nc.vector.tensor_mul(
                    otj[j][0:64, cc, :], otp[0:64, :], lbA[:])
            else:
                # otp rows: [l(0:64) | O(64:128)]
                nc.vector.reciprocal_approx_fast(lbA[:], otp[0:64, :])
                nc.vector.tensor_mul(
                    otj[j][64:128, cc, :], otp[64:128, :], lbA[:])

        def close(gi):
            pvs(gi)
            j, h, _, last = groups[gi]
            if last:
                normalize(j, h)
                if h == HL - 1:
                    post_j(j)

        scores(0)
        for gi in range(1, len(groups)):
            scores(gi)
            close(gi - 1)
        close(len(groups) - 1)

    # ---- output projection: per chunk, own 2-bank ring ----
    def emit_proj(j):
        for tbl in range(4):
            po = pout.tile([128, C], F32, tag="po")
            for n2 in range(2):
                ps = pp.tile([128, 512], F32, tag="pj", bufs=2)
                for cc in range(2):
                    nc.tensor.matmul(
                        ps[:],
                        lhsT=otj[j][:, cc, tbl * 128:(tbl + 1) * 128],
                        rhs=wpT[:, cc, n2 * 512:(n2 + 1) * 512],
                        start=(cc == 0), stop=(cc == 1))
                nc.vector.tensor_copy(po[:, n2 * 512:(n2 + 1) * 512], ps[:])
            tb = 4 * j + tbl
            nc.sync.dma_start(p_d[tb * 128:(tb + 1) * 128, :], po[:])

    emit_c_wave(0)
    emit_c_wave(1)
    emit_b(0)
    emit_b(2)
    emit_b(1)
    emit_b(3)
    emit_attention(post_j=emit_proj)


_NC_CACHE = None


def _get_nc():
    global _NC_CACHE
    if _NC_CACHE is None:
        _NC_CACHE = build_nc()
    return _NC_CACHE


def make_in_maps(x, w_qkv, w_proj):
    x = np.asarray(x, np.float32)
    w_qkv = np.asarray(w_qkv, np.float32)
    w_proj = np.asarray(w_proj, np.float32)
    bf = ml_dtypes.bfloat16
    in_maps = []
    for c in range(8):
        b, g = divmod(c, 4)
        wq = w_qkv[g * 256:(g + 1) * 256] * 0.125  # fold 1/sqrt(Dh)
        wk = w_qkv[C + g * 256:C + (g + 1) * 256]
        wv = w_qkv[2 * C + g * 256:2 * C + (g + 1) * 256]
        wqk = np.concatenate([wq, wk], 0)  # [512, C]
        in_maps.append({
            "xt": np.ascontiguousarray(x[b].T).astype(bf),
            "wqkt": np.ascontiguousarray(wqk.T).astype(bf),
            "wvt": np.ascontiguousarray(wv.T).astype(bf),
            "wpt": np.ascontiguousarray(w_proj[:, g * 256:(g + 1) * 256].T).astype(bf),
        })
    return in_maps


def combine(results):
    return np.stack(
        [results[4 * b]["p"] + results[4 * b + 1]["p"]
         + results[4 * b + 2]["p"] + results[4 * b + 3]["p"]
         for b in range(2)], 0)


def kernel(x, w_qkv, w_proj):
    nc = _get_nc()
    res = bass_utils.run_bass_kernel_spmd(
        nc, make_in_maps(x, w_qkv, w_proj), core_ids=list(range(8)))
    return combine(res.results)
